# revision 8
# baseline (speedup 1.0000x reference)
"""Trainium2 Bass kernel for CandidateRepresentationLayer (segment span-mean).

Strategy (data-parallel over batch, per sharding hint):
  - core c owns batches [4c, 4c+4); candidates routed to cores by batch_idx.
  - span mean over word_repr[b, sid:eid) is computed as a one-hot matmul:
    a span matrix W (built on host from sid/eid) times a 64-row window of
    word_repr held in SBUF.  Windows start every 57 rows; spans have length
    <= 8, so every span fits in the window containing its sid.
  - word is shipped as a bf16 hi/lo split (word = hi + lo, exact to ~2^-17
    relative).  hi rows sit on SBUF partitions 0..63 and lo rows on 64..127
    of each window tile, and the span matrix rows are duplicated, so ONE
    K=128 bf16 matmul per PSUM half contracts hi+lo together — f32-accurate
    to ~1e-5 at full bf16 PE speed.
  - candidates are bucketed by (batch-position-in-core, window); bucket
    capacity = max count over cores rounded to 32, so all 8 cores run one
    identical (SPMD) instruction stream.  Within each core, batches are
    assigned to positions in decreasing candidate-count order, which aligns
    the per-core distributions and trims the shared capacity.  Buckets are
    cut into matmul blocks of up to 128 candidates; tail blocks are narrower
    to cut padded output DMA.
  - after the matmul, a per-partition scalar multiply by 1/len (alternating
    vector/scalar engines) moves PSUM -> SBUF, then DMA writes the block.
    Host inverse-permutes rows back to candidate order and computes the tiny
    index outputs (label/counts/valid/loc) directly.
"""

import numpy as np

_B, _S, _D = 32, 512, 1024
_A, _N = 4, 65536
_NCORES, _BPC = 8, 4          # cores, batches per core
_WROWS = 64                   # window rows (hi + lo stacked -> 128 partitions)
_WSTEP = _WROWS - 8           # window start spacing; span always fits
_WINS = tuple(range(0, _S - 8, _WSTEP))
_NW = len(_WINS)
_BLK = 128                    # max candidates per matmul block
_GRAN = 32                    # bucket capacity granularity
_CHUNK_SLOTS = 1024           # W-matrix DMA chunk size (slots)
_NTILES = _BPC * _NW          # word window tiles resident in SBUF

_TRACE = False                # test harness may flip this for profiling
LAST_RESULTS = None


def _plan(cap_slots):
    """Cut bucket capacities into blocks and W-DMA chunks.

    Returns (nslot, blocks, chunks); blocks = (slot0, m, bucket);
    chunks = (slot0, nslots, [block ids]).
    """
    blocks = []
    off = 0
    for bu, cap in enumerate(cap_slots):
        rem = int(cap)
        while rem > 0:
            m = min(_BLK, rem)
            blocks.append((off, m, bu))
            off += m
            rem -= m
    nslot = off
    chunks = []
    cur = []
    c0 = 0
    cs = 0
    for bi, (s0, m, _) in enumerate(blocks):
        if cs + m > _CHUNK_SLOTS and cur:
            chunks.append((c0, cs, cur))
            c0, cs, cur = s0, 0, []
        cur.append(bi)
        cs += m
    if cur:
        chunks.append((c0, cs, cur))
    return nslot, blocks, chunks


def _build_program(nslot, blocks, chunks):
    import concourse.tile as tile
    from concourse import bacc, mybir

    f32 = mybir.dt.float32
    bf16 = mybir.dt.bfloat16
    nblock = len(blocks)

    nc = bacc.Bacc(
        "TRN2",
        target_bir_lowering=False,
        debug=False,
        enable_asserts=False,
        num_devices=_NCORES,
    )
    # [p, (tile, d)] — partition p<64: hi row p; p>=64: lo row p-64
    word_d = nc.dram_tensor("word", [_BLK, _NTILES * _D], bf16, kind="ExternalInput")
    wmat_d = nc.dram_tensor("wmat", [_BLK, nslot], bf16, kind="ExternalInput")
    inv_d = nc.dram_tensor("invlen", [_BLK, nblock], f32, kind="ExternalInput")
    out_d = nc.dram_tensor("out", [nslot, _D], f32, kind="ExternalOutput")

    with tile.TileContext(nc) as tc:
        with (
            tc.tile_pool(name="persist", bufs=1) as persist,
            tc.tile_pool(name="wpool", bufs=3) as wpool,
            tc.tile_pool(name="opool", bufs=4) as opool,
            tc.tile_pool(name="psum", bufs=3, space="PSUM") as pp,
            tc.tile_pool(name="pwarm", bufs=1, space="PSUM") as pw,
        ):
            win_sb = persist.tile([_BLK, _NTILES * _D], bf16)
            inv_sb = persist.tile([_BLK, nblock], f32)
            nc.sync.dma_start(inv_sb[:], inv_d[:])

            wts = {}

            def load_chunk(i):
                c0, cs, _ = chunks[i]
                wt = wpool.tile([_BLK, _CHUNK_SLOTS], bf16, tag="wt")
                nc.sync.dma_start(wt[:, :cs], wmat_d[:, c0 : c0 + cs])
                wts[i] = wt

            # Split the window upload so compute starts after the first tile;
            # prefetch the first W chunks between windows.  After each window
            # lands, run a throwaway matmul on it to keep the PE ticking
            # through the load phase (HAM stays un-throttled).
            warm = pw.tile([_BLK, 512], f32, tag="warm")
            load_chunk(0)
            for t in range(_NTILES):
                o = t * _D
                nc.sync.dma_start(win_sb[:, o : o + _D], word_d[:, o : o + _D])
                if t == 0:
                    load_chunk(1)
                nc.tensor.matmul(
                    warm[:],
                    win_sb[:, o : o + _BLK],
                    win_sb[:, o : o + 512],
                    start=True,
                    stop=True,
                )

            for ci, (c0, cs, bids) in enumerate(chunks):
                if ci not in wts:
                    load_chunk(ci)
                wt = wts.pop(ci)
                for bi in bids:
                    s0, m, bu = blocks[bi]
                    ps = pp.tile([_BLK, _D], f32, tag="ps")
                    lhsT = wt[:, s0 - c0 : s0 - c0 + m]
                    base = bu * _D
                    for h in range(2):
                        o = base + h * 512
                        nc.tensor.matmul(
                            ps[:m, h * 512 : (h + 1) * 512],
                            lhsT,
                            win_sb[:, o : o + 512],
                            start=True,
                            stop=True,
                        )
                    ob = opool.tile([_BLK, _D], f32, tag="ob")
                    sc = inv_sb[:m, bi : bi + 1]
                    if bi % 2 == 0:
                        nc.vector.tensor_scalar_mul(ob[:m], ps[:m], sc)
                    else:
                        nc.scalar.mul(ob[:m], ps[:m], sc)
                    nc.sync.dma_start(out_d[s0 : s0 + m, :], ob[:m])
    nc.compile()
    return nc


def kernel(word_repr, candidates_idx, anchor_loc, anchor_cls):
    global LAST_RESULTS
    word_repr = np.asarray(word_repr, dtype=np.float32)
    candidates_idx = np.asarray(candidates_idx)
    anchor_loc = np.asarray(anchor_loc)
    anchor_cls = np.asarray(anchor_cls)

    b = candidates_idx[:, 0].astype(np.int64)
    w = candidates_idx[:, 1].astype(np.int64)
    a = candidates_idx[:, 2].astype(np.int64)
    loc = anchor_loc[b, w, a]  # [N, 2]
    sid = loc[:, 0].astype(np.int64)
    eid = loc[:, 1].astype(np.int64)
    ln = eid - sid
    valid = ln > 0

    # small outputs (pure index gathers)
    label = np.where(valid, anchor_cls[b, w, a], -1).astype(anchor_cls.dtype)
    counts = np.bincount(b[valid], minlength=_B).astype(np.int32)
    loc_out = np.where(valid[:, None], loc, 0).astype(anchor_loc.dtype)

    # --- shard candidates: (core, batch position, window) buckets ---
    core = b // _BPC
    wins = np.asarray(_WINS, dtype=np.int64)
    wj = np.searchsorted(wins, sid, side="right") - 1

    # within each core, order batches by candidate count (descending) so the
    # per-position count distributions align across cores -> lower max
    bcnt = np.bincount(b, minlength=_B)
    lbpos_of_batch = np.zeros(_B, np.int64)
    batch_at_pos = np.zeros((_NCORES, _BPC), np.int64)
    for c in range(_NCORES):
        bs = np.arange(c * _BPC, (c + 1) * _BPC)
        order_b = bs[np.argsort(-bcnt[bs], kind="stable")]
        for pos, bb in enumerate(order_b):
            lbpos_of_batch[bb] = pos
            batch_at_pos[c, pos] = bb

    lb = lbpos_of_batch[b]
    bucket = lb * _NW + wj
    nbuck = _BPC * _NW

    cnt = np.zeros((_NCORES, nbuck), np.int64)
    np.add.at(cnt, (core, bucket), 1)
    cap_slots = ((cnt.max(axis=0) + _GRAN - 1) // _GRAN) * _GRAN
    nslot, blocks, chunks = _plan(cap_slots)
    nblock = len(blocks)

    bstart = np.zeros(nbuck, np.int64)  # bucket start slot
    bstart[1:] = np.cumsum(cap_slots)[:-1]

    # per-core slot assignment
    keys = core * nbuck + bucket
    order = np.argsort(keys, kind="stable")
    sorted_keys = keys[order]
    slot_orig = np.full((_NCORES, nslot), -1, np.int64)
    for c in range(_NCORES):
        lo_i = np.searchsorted(sorted_keys, c * nbuck)
        for bu in range(nbuck):
            hi_i = np.searchsorted(sorted_keys, c * nbuck + bu + 1)
            idxs = order[lo_i:hi_i]
            slot_orig[c, bstart[bu] : bstart[bu] + len(idxs)] = idxs
            lo_i = hi_i

    # --- per-core device inputs ---
    import ml_dtypes

    bf16 = ml_dtypes.bfloat16
    ks = np.arange(_WROWS, dtype=np.int64)
    inv_all = np.where(valid, 1.0 / np.maximum(ln, 1), 0.0).astype(np.float32)
    win_start = wins[wj]  # per candidate

    word_hi = word_repr.astype(bf16)
    word_lo = (word_repr - word_hi.astype(np.float32)).astype(bf16)

    in_maps = []
    for c in range(_NCORES):
        so = slot_orig[c]
        has = so >= 0
        soc = np.where(has, so, 0)
        lo = np.where(has, sid[soc] - win_start[soc], 1)
        hi = np.where(has & valid[soc], eid[soc] - win_start[soc], 0)
        # W [k, slot], k rows 0:64 = span mask, 64:128 = same mask (lo part)
        whalf = (ks[:, None] >= lo[None, :]) & (ks[:, None] < hi[None, :])
        wmat = np.ascontiguousarray(
            np.concatenate([whalf, whalf], axis=0).astype(bf16)
        )

        inv_slot = np.where(has, inv_all[soc], 0.0).astype(np.float32)
        invc = np.zeros((_BLK, nblock), np.float32)
        for bi, (s0, m, _) in enumerate(blocks):
            invc[:m, bi] = inv_slot[s0 : s0 + m]

        tiles = np.zeros((_BPC, _NW, _BLK, _D), bf16)
        for pos in range(_BPC):
            bb = batch_at_pos[c, pos]
            for j, s0 in enumerate(_WINS):
                n = min(_WROWS, _S - s0)
                tiles[pos, j, :n, :] = word_hi[bb, s0 : s0 + n, :]
                tiles[pos, j, _WROWS : _WROWS + n, :] = word_lo[bb, s0 : s0 + n, :]
        # -> [p, (pos, wj, d)]
        word_in = np.ascontiguousarray(
            tiles.transpose(2, 0, 1, 3).reshape(_BLK, _NTILES * _D)
        )
        in_maps.append({"word": word_in, "wmat": wmat, "invlen": invc})

    # --- build, run, unshard ---
    nc = _build_program(nslot, blocks, chunks)
    from concourse.bass_utils import run_bass_kernel_spmd

    res = run_bass_kernel_spmd(
        nc, in_maps, core_ids=list(range(_NCORES)), trace=_TRACE
    )
    LAST_RESULTS = res

    repr_ = np.zeros((_N, _D), np.float32)
    for c in range(_NCORES):
        oc = res.results[c]["out"]
        so = slot_orig[c]
        m = so >= 0
        repr_[so[m]] = oc[m]
    return repr_, label, counts, valid, loc_out


# revision 15
# speedup vs baseline: 1.1794x; 1.1794x over previous
"""Trainium2 Bass kernel for CandidateRepresentationLayer (segment span-mean).

Strategy (data-parallel over batch, per sharding hint):
  - core c owns batches [4c, 4c+4); candidates routed to cores by batch_idx.
  - span mean over word_repr[b, sid:eid) is computed as a one-hot matmul:
    a span matrix W (built on host from sid/eid) times a 64-row window of
    word_repr held in SBUF.  Windows start every 57 rows; spans have length
    <= 8, so every span fits in the window containing its sid.
  - word is shipped as a bf16 hi/lo split (word = hi + lo, exact to ~2^-17
    relative).  hi rows sit on SBUF partitions 0..63 and lo rows on 64..127
    of each window tile, and the span matrix rows are duplicated, so ONE
    K=128 bf16 matmul per PSUM half contracts hi+lo together — f32-accurate
    to ~1e-5 at full bf16 PE speed.
  - candidates are bucketed by (batch-position-in-core, window); bucket
    capacity = max count over cores rounded to 32, so all 8 cores run one
    identical (SPMD) instruction stream.  Within each core, batches are
    assigned to positions in decreasing candidate-count order, which aligns
    the per-core distributions and trims the shared capacity.  Buckets are
    cut into matmul blocks of up to 128 candidates; tail blocks are narrower
    to cut padded output DMA.
  - after the matmul, a per-partition scalar multiply by 1/len (alternating
    vector/scalar engines) moves PSUM -> SBUF, then DMA writes the block.
    Host inverse-permutes rows back to candidate order and computes the tiny
    index outputs (label/counts/valid/loc) directly.
"""

import numpy as np

_B, _S, _D = 32, 512, 1024
_A, _N = 4, 65536
_NCORES, _BPC = 8, 4          # cores, batches per core
_WROWS = 64                   # window rows (hi + lo stacked -> 128 partitions)
_WSTEP = _WROWS - 8           # window start spacing; span always fits
_WINS = tuple(range(0, _S - 8, _WSTEP))
_NW = len(_WINS)
_BLK = 128                    # max candidates per matmul block
_GRAN = 32                    # bucket capacity granularity
_CHUNK_SLOTS = 2048           # W-matrix DMA chunk size (slots)
_OGROUP = 2                   # full blocks per output DMA
_NTILES = _BPC * _NW          # word window tiles resident in SBUF

_TRACE = False                # test harness may flip this for profiling
LAST_RESULTS = None


def _plan(cap_slots):
    """Cut bucket capacities into blocks and W-DMA chunks.

    Returns (nslot, blocks, chunks); blocks = (slot0, m, bucket);
    chunks = (slot0, nslots, [block ids]).
    """
    blocks = []
    off = 0
    for bu, cap in enumerate(cap_slots):
        rem = int(cap)
        while rem > 0:
            m = min(_BLK, rem)
            blocks.append((off, m, bu))
            off += m
            rem -= m
    nslot = off
    chunks = []
    cur = []
    c0 = 0
    cs = 0
    for bi, (s0, m, _) in enumerate(blocks):
        if cs + m > _CHUNK_SLOTS and cur:
            chunks.append((c0, cs, cur))
            c0, cs, cur = s0, 0, []
        cur.append(bi)
        cs += m
    if cur:
        chunks.append((c0, cs, cur))
    # group consecutive full blocks for combined output DMAs
    ogroups = []
    i = 0
    while i < len(blocks):
        g = [i]
        while (
            len(g) < _OGROUP
            and i + 1 < len(blocks)
            and blocks[i][1] == _BLK
            and blocks[i + 1][1] == _BLK
        ):
            i += 1
            g.append(i)
        ogroups.append(g)
        i += 1
    return nslot, blocks, chunks, ogroups


def _build_program(nslot, blocks, chunks, ogroups):
    import concourse.tile as tile
    from concourse import bacc, mybir

    f32 = mybir.dt.float32
    bf16 = mybir.dt.bfloat16
    nblock = len(blocks)

    nc = bacc.Bacc(
        "TRN2",
        target_bir_lowering=False,
        debug=False,
        enable_asserts=False,
        num_devices=_NCORES,
    )
    # [p, (tile, d)] — partition p<64: hi row p; p>=64: lo row p-64
    word_d = nc.dram_tensor("word", [_BLK, _NTILES * _D], bf16, kind="ExternalInput")
    wmat_d = nc.dram_tensor("wmat", [_BLK, nslot], bf16, kind="ExternalInput")
    inv_d = nc.dram_tensor("invlen", [_BLK, nblock], f32, kind="ExternalInput")
    out_d = nc.dram_tensor("out", [nslot, _D], f32, kind="ExternalOutput")

    with tile.TileContext(nc) as tc:
        with (
            tc.tile_pool(name="persist", bufs=1) as persist,
            tc.tile_pool(name="wpool", bufs=3) as wpool,
            tc.tile_pool(name="opool", bufs=4) as opool,
            tc.tile_pool(name="psum", bufs=3, space="PSUM") as pp,
            tc.tile_pool(name="pwarm", bufs=1, space="PSUM") as pw,
        ):
            win_sb = persist.tile([_BLK, _NTILES * _D], bf16)
            inv_sb = persist.tile([_BLK, nblock], f32)
            nc.sync.dma_start(inv_sb[:], inv_d[:])

            wts = {}

            def load_chunk(i):
                c0, cs, _ = chunks[i]
                wt = wpool.tile([_BLK, _CHUNK_SLOTS], bf16, tag="wt")
                nc.sync.dma_start(wt[:, :cs], wmat_d[:, c0 : c0 + cs])
                wts[i] = wt

            # Split the window upload (first batch-position in 3 pieces for a
            # fast start, the rest in one DMA each); prefetch the first W
            # chunks between windows.  After each piece lands, run a
            # throwaway matmul on it to keep the PE ticking through the load
            # phase (HAM stays un-throttled).
            warm = pw.tile([_BLK, 512], f32, tag="warm")
            load_chunk(0)
            for pos in range(_BPC):
                o = pos * _NW * _D
                pieces = 3 if pos == 0 else 1
                step = _NW * _D // pieces
                for piece in range(pieces):
                    po = o + piece * step
                    nc.sync.dma_start(
                        win_sb[:, po : po + step], word_d[:, po : po + step]
                    )
                    nc.tensor.matmul(
                        warm[:],
                        win_sb[:, po : po + _BLK],
                        win_sb[:, po : po + 512],
                        start=True,
                        stop=True,
                    )
                if pos == 0:
                    load_chunk(1)

            chunk_of = {}
            for ci, (_, _, bids) in enumerate(chunks):
                for bi in bids:
                    chunk_of[bi] = ci

            for g in ogroups:
                ob = opool.tile([_BLK, _OGROUP, _D], f32, tag="ob")
                for gi, bi in enumerate(g):
                    ci = chunk_of[bi]
                    if ci not in wts:
                        load_chunk(ci)
                    wt = wts[ci]
                    c0 = chunks[ci][0]
                    s0, m, bu = blocks[bi]
                    ps = pp.tile([_BLK, _D], f32, tag="ps")
                    lhsT = wt[:, s0 - c0 : s0 - c0 + m]
                    base = bu * _D
                    for h in range(2):
                        o = base + h * 512
                        nc.tensor.matmul(
                            ps[:m, h * 512 : (h + 1) * 512],
                            lhsT,
                            win_sb[:, o : o + 512],
                            start=True,
                            stop=True,
                        )
                    sc = inv_sb[:m, bi : bi + 1]
                    od = ob[:m, gi, :]
                    if bi % 2 == 0:
                        nc.vector.tensor_scalar_mul(od, ps[:m], sc)
                    else:
                        nc.scalar.mul(od, ps[:m], sc)
                gs0 = blocks[g[0]][0]
                if len(g) == _OGROUP:
                    dap = out_d[gs0 : gs0 + _OGROUP * _BLK, :].rearrange(
                        "(two p) d -> p two d", two=_OGROUP
                    )
                    nc.sync.dma_start(dap, ob[:])
                else:
                    s0, m, _ = blocks[g[0]]
                    nc.sync.dma_start(out_d[s0 : s0 + m, :], ob[:m, 0, :])
    nc.compile()
    return nc


def kernel(word_repr, candidates_idx, anchor_loc, anchor_cls):
    global LAST_RESULTS
    word_repr = np.asarray(word_repr, dtype=np.float32)
    candidates_idx = np.asarray(candidates_idx)
    anchor_loc = np.asarray(anchor_loc)
    anchor_cls = np.asarray(anchor_cls)

    b = candidates_idx[:, 0].astype(np.int64)
    w = candidates_idx[:, 1].astype(np.int64)
    a = candidates_idx[:, 2].astype(np.int64)
    loc = anchor_loc[b, w, a]  # [N, 2]
    sid = loc[:, 0].astype(np.int64)
    eid = loc[:, 1].astype(np.int64)
    ln = eid - sid
    valid = ln > 0

    # small outputs (pure index gathers)
    label = np.where(valid, anchor_cls[b, w, a], -1).astype(anchor_cls.dtype)
    counts = np.bincount(b[valid], minlength=_B).astype(np.int32)
    loc_out = np.where(valid[:, None], loc, 0).astype(anchor_loc.dtype)

    # --- shard candidates: (core, batch position, window) buckets ---
    core = b // _BPC
    wins = np.asarray(_WINS, dtype=np.int64)
    wj = np.searchsorted(wins, sid, side="right") - 1

    # within each core, order batches by candidate count (descending) so the
    # per-position count distributions align across cores -> lower max
    bcnt = np.bincount(b, minlength=_B)
    lbpos_of_batch = np.zeros(_B, np.int64)
    batch_at_pos = np.zeros((_NCORES, _BPC), np.int64)
    for c in range(_NCORES):
        bs = np.arange(c * _BPC, (c + 1) * _BPC)
        order_b = bs[np.argsort(-bcnt[bs], kind="stable")]
        for pos, bb in enumerate(order_b):
            lbpos_of_batch[bb] = pos
            batch_at_pos[c, pos] = bb

    lb = lbpos_of_batch[b]
    bucket = lb * _NW + wj
    nbuck = _BPC * _NW

    cnt = np.zeros((_NCORES, nbuck), np.int64)
    np.add.at(cnt, (core, bucket), 1)
    cap_slots = ((cnt.max(axis=0) + _GRAN - 1) // _GRAN) * _GRAN
    nslot, blocks, chunks, ogroups = _plan(cap_slots)
    nblock = len(blocks)

    bstart = np.zeros(nbuck, np.int64)  # bucket start slot
    bstart[1:] = np.cumsum(cap_slots)[:-1]

    # per-core slot assignment
    keys = core * nbuck + bucket
    order = np.argsort(keys, kind="stable")
    sorted_keys = keys[order]
    slot_orig = np.full((_NCORES, nslot), -1, np.int64)
    for c in range(_NCORES):
        lo_i = np.searchsorted(sorted_keys, c * nbuck)
        for bu in range(nbuck):
            hi_i = np.searchsorted(sorted_keys, c * nbuck + bu + 1)
            idxs = order[lo_i:hi_i]
            slot_orig[c, bstart[bu] : bstart[bu] + len(idxs)] = idxs
            lo_i = hi_i

    # --- per-core device inputs ---
    import ml_dtypes

    bf16 = ml_dtypes.bfloat16
    ks = np.arange(_WROWS, dtype=np.int64)
    inv_all = np.where(valid, 1.0 / np.maximum(ln, 1), 0.0).astype(np.float32)
    win_start = wins[wj]  # per candidate

    word_hi = word_repr.astype(bf16)
    word_lo = (word_repr - word_hi.astype(np.float32)).astype(bf16)

    in_maps = []
    for c in range(_NCORES):
        so = slot_orig[c]
        has = so >= 0
        soc = np.where(has, so, 0)
        lo = np.where(has, sid[soc] - win_start[soc], 1)
        hi = np.where(has & valid[soc], eid[soc] - win_start[soc], 0)
        # W [k, slot], k rows 0:64 = span mask, 64:128 = same mask (lo part)
        whalf = (ks[:, None] >= lo[None, :]) & (ks[:, None] < hi[None, :])
        wmat = np.ascontiguousarray(
            np.concatenate([whalf, whalf], axis=0).astype(bf16)
        )

        inv_slot = np.where(has, inv_all[soc], 0.0).astype(np.float32)
        invc = np.zeros((_BLK, nblock), np.float32)
        for bi, (s0, m, _) in enumerate(blocks):
            invc[:m, bi] = inv_slot[s0 : s0 + m]

        tiles = np.zeros((_BPC, _NW, _BLK, _D), bf16)
        for pos in range(_BPC):
            bb = batch_at_pos[c, pos]
            for j, s0 in enumerate(_WINS):
                n = min(_WROWS, _S - s0)
                tiles[pos, j, :n, :] = word_hi[bb, s0 : s0 + n, :]
                tiles[pos, j, _WROWS : _WROWS + n, :] = word_lo[bb, s0 : s0 + n, :]
        # -> [p, (pos, wj, d)]
        word_in = np.ascontiguousarray(
            tiles.transpose(2, 0, 1, 3).reshape(_BLK, _NTILES * _D)
        )
        in_maps.append({"word": word_in, "wmat": wmat, "invlen": invc})

    # --- build, run, unshard ---
    nc = _build_program(nslot, blocks, chunks, ogroups)
    from concourse.bass_utils import run_bass_kernel_spmd

    res = run_bass_kernel_spmd(
        nc, in_maps, core_ids=list(range(_NCORES)), trace=_TRACE
    )
    LAST_RESULTS = res

    repr_ = np.zeros((_N, _D), np.float32)
    for c in range(_NCORES):
        oc = res.results[c]["out"]
        so = slot_orig[c]
        m = so >= 0
        repr_[so[m]] = oc[m]
    return repr_, label, counts, valid, loc_out
